# revision 5
# baseline (speedup 1.0000x reference)
"""DeepFM forward on Trainium2, 8 NeuronCores, data-parallel over batch.

Reference computes (B=512, n=512, K=4, H=128, n_pairs=130816):
    S  = fm_w @ fm_w.T
    fm = x[:, i1] * x[:, i2] * S[i1, i2]        # [B, n_pairs]
    h2 = relu(relu(x@w1+b1)@w2+b2)
    out = sigmoid(concat([fm, h2]) @ wo + bo)

The fm @ wo[:n_pairs] contraction is the bilinear form
    t1[b] = x[b]^T A x[b],   A[i,j] = S[i,j] * Wp[i,j]  (strictly upper)
where Wp is wo[:n_pairs] scattered into the upper triangle of a [n, n]
matrix (a pure re-layout of wo done on host; indices are static). The
device kernel therefore only needs one 512x512 matmul + row reduction per
batch shard instead of 130k-wide gathers.

Per-core program (batch shard = 64 columns, feature-on-partition layout):
    S_m  = fm_wT[:, m128]^T @ fm_wT              (PE, K=4)      [128, 512]
    A_m  = S_m * Wp_m                            (DVE)          -> SBUF
    YT_j = sum_k A[k128, j128]^T @ xT_k          (PE)           = (x@A)^T
    Q_j  = YT_j * xT_j                           (DVE)
    h1   = relu(w1^T @ xT + b1)                  (PE+ACT)       [128, 64]
    h2   = relu(w2^T @ h1 + b2)                  (PE+ACT)       [128, 64]
    t    = sum_j ones^T @ Q_j + wo_h^T @ h2      (PE psum accumulation) [1, 64]
    out  = sigmoid(t + bo)                       (ACT)
"""

import os
import sys

import numpy as np

for _p in ("/opt/trn_rl_repo", "/root/.axon_site/_ro/trn_rl_repo"):
    if os.path.isdir(_p) and _p not in sys.path:
        sys.path.insert(0, _p)

import concourse.bass as bass
import concourse.tile as tile
from concourse import bacc, mybir
from concourse.bass import ts
from concourse.bass_utils import run_bass_kernel_spmd

F32 = mybir.dt.float32
AF = mybir.ActivationFunctionType

N = 512          # n_feat
KFM = 4          # fm embedding dim
H = 128          # mlp hidden
NP = N * (N - 1) // 2
B = 512
N_CORES = 8
BC = B // N_CORES  # 64 batch rows per core
NCH = N // 128     # 4 feature chunks

_IU1, _IU2 = np.triu_indices(N, k=1)

_program_cache = None


def _build_program():
    global _program_cache
    if _program_cache is not None:
        return _program_cache

    nc = bacc.Bacc(
        "TRN2", target_bir_lowering=False, debug=False, num_devices=N_CORES
    )
    xT_d = nc.declare_dram_parameter("xT", [N, BC], F32, isOutput=False)
    wp_d = nc.declare_dram_parameter("wp", [N, N], F32, isOutput=False)
    fmwT_d = nc.declare_dram_parameter("fmwT", [KFM, N], F32, isOutput=False)
    w1_d = nc.declare_dram_parameter("w1", [N, H], F32, isOutput=False)
    w2_d = nc.declare_dram_parameter("w2", [H, H], F32, isOutput=False)
    woh_d = nc.declare_dram_parameter("woh", [H, 1], F32, isOutput=False)
    b1_d = nc.declare_dram_parameter("b1", [H, 1], F32, isOutput=False)
    b2_d = nc.declare_dram_parameter("b2", [H, 1], F32, isOutput=False)
    bo_d = nc.declare_dram_parameter("bo", [1, 1], F32, isOutput=False)
    out_d = nc.declare_dram_parameter("out", [1, BC], F32, isOutput=True)

    with tile.TileContext(nc) as tc:
        with (
            tc.tile_pool(name="const", bufs=1) as cpool,
            tc.tile_pool(name="work", bufs=2) as wpool,
            tc.tile_pool(name="ps_s", bufs=2, space=bass.MemorySpace.PSUM) as spool,
            tc.tile_pool(name="ps_y", bufs=2, space=bass.MemorySpace.PSUM) as ypool,
            tc.tile_pool(name="ps_h", bufs=1, space=bass.MemorySpace.PSUM) as hpool,
            tc.tile_pool(name="ps_t", bufs=1, space=bass.MemorySpace.PSUM) as tpool,
        ):
            # ---- loads ----
            xt_sb = cpool.tile([128, NCH, BC], F32)
            nc.sync.dma_start(
                xt_sb[:], xT_d[:, :].rearrange("(c p) b -> p c b", p=128)
            )
            fmw_sb = cpool.tile([KFM, N], F32)
            nc.sync.dma_start(fmw_sb[:], fmwT_d[:, :])
            wp_sb = cpool.tile([128, NCH, N], F32)
            nc.sync.dma_start(
                wp_sb[:], wp_d[:, :].rearrange("(c p) j -> p c j", p=128)
            )
            w1_sb = cpool.tile([128, NCH, H], F32)
            nc.sync.dma_start(
                w1_sb[:], w1_d[:, :].rearrange("(c p) h -> p c h", p=128)
            )
            w2_sb = cpool.tile([H, H], F32)
            nc.sync.dma_start(w2_sb[:], w2_d[:, :])
            woh_sb = cpool.tile([H, 1], F32)
            nc.sync.dma_start(woh_sb[:], woh_d[:, :])
            b1_sb = cpool.tile([H, 1], F32)
            nc.sync.dma_start(b1_sb[:], b1_d[:, :])
            b2_sb = cpool.tile([H, 1], F32)
            nc.sync.dma_start(b2_sb[:], b2_d[:, :])
            bo_sb = cpool.tile([1, 1], F32)
            nc.sync.dma_start(bo_sb[:], bo_d[:, :])
            ones_sb = cpool.tile([128, 1], F32)
            nc.gpsimd.memset(ones_sb[:], 1.0)

            # ---- MLP (independent of the big Wp load) ----
            h1_ps = hpool.tile([H, BC], F32)
            for k in range(NCH):
                nc.tensor.matmul(
                    h1_ps[:], w1_sb[:, k, :], xt_sb[:, k, :],
                    start=(k == 0), stop=(k == NCH - 1),
                )
            h1_sb = wpool.tile([H, BC], F32)
            nc.scalar.activation(h1_sb[:], h1_ps[:], AF.Relu, bias=b1_sb[:])

            h2_ps = hpool.tile([H, BC], F32)
            nc.tensor.matmul(h2_ps[:], w2_sb[:], h1_sb[:], start=True, stop=True)
            h2_sb = wpool.tile([H, BC], F32)
            nc.scalar.activation(h2_sb[:], h2_ps[:], AF.Relu, bias=b2_sb[:])

            # ---- A = (fm_w @ fm_w.T) * Wp, built row-chunk by row-chunk ----
            a_sb = cpool.tile([128, NCH, N], F32)
            for m in range(NCH):
                s_ps = spool.tile([128, N], F32)
                nc.tensor.matmul(
                    s_ps[:], fmw_sb[:, ts(m, 128)], fmw_sb[:, :],
                    start=True, stop=True,
                )
                nc.vector.tensor_mul(a_sb[:, m, :], s_ps[:], wp_sb[:, m, :])

            # ---- YT = (x @ A)^T and Q = YT * xT ----
            q_tiles = []
            for j in range(NCH):
                yt_ps = ypool.tile([128, BC], F32)
                for k in range(NCH):
                    nc.tensor.matmul(
                        yt_ps[:], a_sb[:, k, ts(j, 128)], xt_sb[:, k, :],
                        start=(k == 0), stop=(k == NCH - 1),
                    )
                q_sb = wpool.tile([128, BC], F32, tag=f"q{j}")
                nc.vector.tensor_mul(q_sb[:], yt_ps[:], xt_sb[:, j, :])
                q_tiles.append(q_sb)

            # ---- logit = sum_j 1^T Q_j + wo_h^T h2, then sigmoid(. + bo) ----
            t_ps = tpool.tile([1, BC], F32)
            for j in range(NCH):
                nc.tensor.matmul(
                    t_ps[:], ones_sb[:], q_tiles[j][:],
                    start=(j == 0), stop=False,
                )
            nc.tensor.matmul(t_ps[:], woh_sb[:], h2_sb[:], start=False, stop=True)

            out_sb = wpool.tile([1, BC], F32)
            nc.scalar.activation(out_sb[:], t_ps[:], AF.Sigmoid, bias=bo_sb[:])
            nc.sync.dma_start(out_d[:, :], out_sb[:])

    nc.compile()
    _program_cache = nc
    return nc


def _prep_inputs(x, fm_w, w1, b1, w2, b2, wo, bo):
    x = np.ascontiguousarray(np.asarray(x, dtype=np.float32))
    fm_w = np.asarray(fm_w, dtype=np.float32)
    w1 = np.ascontiguousarray(np.asarray(w1, dtype=np.float32))
    w2 = np.ascontiguousarray(np.asarray(w2, dtype=np.float32))
    wo = np.asarray(wo, dtype=np.float32).reshape(NP + H)
    b1 = np.asarray(b1, dtype=np.float32).reshape(H, 1)
    b2 = np.asarray(b2, dtype=np.float32).reshape(H, 1)
    bo = np.asarray(bo, dtype=np.float32).reshape(1, 1)

    # Scatter the pair weights into the strictly-upper triangle (static
    # index relayout; same (j1, j2>j1) row-major order as the reference).
    wp = np.zeros((N, N), dtype=np.float32)
    wp[_IU1, _IU2] = wo[:NP]
    woh = np.ascontiguousarray(wo[NP:]).reshape(H, 1)

    fmwT = np.ascontiguousarray(fm_w.T)  # [4, 512]
    xT = np.ascontiguousarray(x.T)       # [512, 512]

    shared = {
        "wp": wp, "fmwT": fmwT, "w1": w1, "w2": w2,
        "woh": woh, "b1": b1, "b2": b2, "bo": np.ascontiguousarray(bo),
    }
    in_maps = []
    for c in range(N_CORES):
        m = dict(shared)
        m["xT"] = np.ascontiguousarray(xT[:, c * BC:(c + 1) * BC])
        in_maps.append(m)
    return in_maps


def run(inputs, **spmd_kwargs):
    """Build, run on 8 cores, return (output [512,1] f32, BassKernelResults)."""
    nc = _build_program()
    in_maps = _prep_inputs(**inputs)
    res = run_bass_kernel_spmd(nc, in_maps, list(range(N_CORES)), **spmd_kwargs)
    out = np.concatenate(
        [res.results[c]["out"].reshape(BC) for c in range(N_CORES)]
    ).reshape(B, 1).astype(np.float32)
    return out, res


def kernel(**inputs) -> np.ndarray:
    out, _ = run(inputs)
    return out


# revision 8
# speedup vs baseline: 1.5282x; 1.5282x over previous
"""DeepFM forward on Trainium2, 8 NeuronCores, data-parallel over batch.

Reference computes (B=512, n=512, K=4, H=128, n_pairs=130816):
    S  = fm_w @ fm_w.T
    fm = x[:, i1] * x[:, i2] * S[i1, i2]        # [B, n_pairs]
    h2 = relu(relu(x@w1+b1)@w2+b2)
    out = sigmoid(concat([fm, h2]) @ wo + bo)

The fm @ wo[:n_pairs] contraction is the bilinear form
    t1[b] = x[b]^T A x[b],   A[i,j] = S[i,j] * Wp[i,j]  (strictly upper)
where Wp is wo[:n_pairs] scattered into the upper triangle of a [n, n]
matrix (a pure re-layout of wo done on host; indices are static). The
device kernel therefore only needs one 512x512 matmul + row reduction per
batch shard instead of 130k-wide gathers.

Per-core program (batch shard = 64 columns, feature-on-partition layout,
bf16 operands / fp32 accumulation):
    S_m  = fm_wT[:, m128]^T @ fm_wT              (PE, K=4)      [128, 512]
    A_m  = S_m * Wp_m                            (DVE)          -> SBUF bf16
    YT_j = sum_k A[k128, j128]^T @ xT_k          (PE, k-major)  = (x@A)^T
    Q_j  = YT_j * xT_j                           (DVE)
    h1   = max(w1^T @ xT + b1, 0)                (PE+DVE)       [128, 64]
    h2   = max(w2^T @ h1 + b2, 0)                (PE+DVE)       [128, 64]
    t    = sum_j ones^T @ Q_j + wo_h^T @ h2      (PE psum accumulation) [1, 64]
    out  = sigmoid(t + bo)                       (ACT, table pre-warmed)
"""

import os
import sys

import numpy as np

for _p in ("/opt/trn_rl_repo", "/root/.axon_site/_ro/trn_rl_repo"):
    if os.path.isdir(_p) and _p not in sys.path:
        sys.path.insert(0, _p)

import ml_dtypes

import concourse.bass as bass
import concourse.tile as tile
from concourse import bacc, mybir
from concourse.bass import ts
from concourse.bass_utils import run_bass_kernel_spmd

F32 = mybir.dt.float32
BF16 = mybir.dt.bfloat16
AF = mybir.ActivationFunctionType

N = 512          # n_feat
KFM = 4          # fm embedding dim
H = 128          # mlp hidden
NP = N * (N - 1) // 2
B = 512
N_CORES = 8
BC = B // N_CORES  # 64 batch rows per core
NCH = N // 128     # 4 feature chunks

_IU1, _IU2 = np.triu_indices(N, k=1)

_program_cache = None


def _build_program():
    global _program_cache
    if _program_cache is not None:
        return _program_cache

    nc = bacc.Bacc(
        "TRN2", target_bir_lowering=False, debug=False, num_devices=N_CORES
    )
    xT_d = nc.declare_dram_parameter("xT", [N, BC], BF16, isOutput=False)
    wp_d = nc.declare_dram_parameter("wp", [N, N], BF16, isOutput=False)
    fmwT_d = nc.declare_dram_parameter("fmwT", [KFM, N], BF16, isOutput=False)
    w1_d = nc.declare_dram_parameter("w1", [N, H], BF16, isOutput=False)
    w2_d = nc.declare_dram_parameter("w2", [H, H], BF16, isOutput=False)
    # pack columns: 0=b1, 1=b2, 2=wo_h, 3=[bo, 0...]
    pack_d = nc.declare_dram_parameter("pack", [H, 4], F32, isOutput=False)
    out_d = nc.declare_dram_parameter("out", [1, BC], F32, isOutput=True)

    with tile.TileContext(nc) as tc:
        with (
            tc.tile_pool(name="const", bufs=1) as cpool,
            tc.tile_pool(name="work", bufs=1) as wpool,
            tc.tile_pool(name="ps_s", bufs=1, space=bass.MemorySpace.PSUM) as spool,
            tc.tile_pool(name="ps_y", bufs=1, space=bass.MemorySpace.PSUM) as ypool,
            tc.tile_pool(name="ps_h", bufs=1, space=bass.MemorySpace.PSUM) as hpool,
            tc.tile_pool(name="ps_t", bufs=1, space=bass.MemorySpace.PSUM) as tpool,
        ):
            # ---- warm the sigmoid ACT table off the critical path ----
            warm_in = cpool.tile([1, 1], F32)
            nc.gpsimd.memset(warm_in[:], 0.0)
            warm_out = cpool.tile([1, 1], F32)
            nc.scalar.activation(warm_out[:], warm_in[:], AF.Sigmoid, bias=0.0)

            ones_sb = cpool.tile([128, 1], F32)
            nc.gpsimd.memset(ones_sb[:], 1.0)

            # ---- loads: sync gets the S->A->YT critical path, scalar the MLP ----
            fmw_sb = cpool.tile([KFM, N], BF16)
            nc.sync.dma_start(fmw_sb[:], fmwT_d[:, :])
            wp_sb = cpool.tile([128, NCH, N], BF16)
            for h in range(2):  # two 256KB issues, halves land staggered
                nc.sync.dma_start(
                    wp_sb[:, ts(h, 2), :],
                    wp_d[:, :].rearrange("(c p) j -> p c j", p=128)[:, ts(h, 2), :],
                )
            xt_sb = cpool.tile([128, NCH, BC], BF16)
            nc.scalar.dma_start(
                xt_sb[:], xT_d[:, :].rearrange("(c p) b -> p c b", p=128)
            )
            pack_sb = cpool.tile([H, 4], F32)
            nc.scalar.dma_start(pack_sb[:], pack_d[:, :])
            w1_sb = cpool.tile([128, NCH, H], BF16)
            nc.scalar.dma_start(
                w1_sb[:], w1_d[:, :].rearrange("(c p) h -> p c h", p=128)
            )
            w2_sb = cpool.tile([H, H], BF16)
            nc.scalar.dma_start(w2_sb[:], w2_d[:, :])

            # ---- S = fm_wT^T @ fm_wT (tiny K=4; overlaps the wp load) ----
            s_tiles = []
            for m in range(NCH):
                s_ps = spool.tile([128, N], F32, tag="s")
                nc.tensor.matmul(
                    s_ps[:], fmw_sb[:, ts(m, 128)], fmw_sb[:, :],
                    start=True, stop=True,
                )
                s_tiles.append(s_ps)

            # ---- A_m = S_m * Wp_m, gated by the wp halves landing ----
            a_sb = cpool.tile([128, NCH, N], BF16)
            for m in range(NCH):
                nc.vector.tensor_mul(a_sb[:, m, :], s_tiles[m][:], wp_sb[:, m, :])

            # ---- MLP on scalar-engine-loaded weights ----
            h1_ps = hpool.tile([H, BC], F32)
            for k in range(NCH):
                nc.tensor.matmul(
                    h1_ps[:], w1_sb[:, k, :], xt_sb[:, k, :],
                    start=(k == 0), stop=(k == NCH - 1),
                )
            h1_sb = wpool.tile([H, BC], BF16)
            nc.vector.tensor_scalar(
                h1_sb[:], h1_ps[:], pack_sb[:, 0:1], 0.0,
                op0=mybir.AluOpType.add, op1=mybir.AluOpType.max,
            )
            h2_ps = hpool.tile([H, BC], F32)
            nc.tensor.matmul(h2_ps[:], w2_sb[:], h1_sb[:], start=True, stop=True)
            h2_sb = wpool.tile([H, BC], F32)
            nc.vector.tensor_scalar(
                h2_sb[:], h2_ps[:], pack_sb[:, 1:2], 0.0,
                op0=mybir.AluOpType.add, op1=mybir.AluOpType.max,
            )

            # ---- YT = (x @ A)^T, k-major so partials start on first A chunk ----
            yt_tiles = [
                ypool.tile([128, BC], F32, name=f"yt{j}", tag=f"y{j}")
                for j in range(NCH)
            ]
            for k in range(NCH):
                for j in range(NCH):
                    nc.tensor.matmul(
                        yt_tiles[j][:], a_sb[:, k, ts(j, 128)], xt_sb[:, k, :],
                        start=(k == 0), stop=(k == NCH - 1),
                    )
            q_tiles = []
            for j in range(NCH):
                q_sb = wpool.tile([128, BC], F32, tag=f"q{j}")
                nc.vector.tensor_mul(q_sb[:], yt_tiles[j][:], xt_sb[:, j, :])
                q_tiles.append(q_sb)

            # ---- logit accumulation and sigmoid ----
            t_ps = tpool.tile([1, BC], F32)
            for j in range(NCH):
                nc.tensor.matmul(
                    t_ps[:], ones_sb[:], q_tiles[j][:],
                    start=(j == 0), stop=False,
                )
            nc.tensor.matmul(
                t_ps[:], pack_sb[:, 2:3], h2_sb[:], start=False, stop=True
            )
            out_sb = wpool.tile([1, BC], F32)
            nc.scalar.activation(
                out_sb[:], t_ps[:], AF.Sigmoid, bias=pack_sb[0:1, 3:4]
            )
            nc.sync.dma_start(out_d[:, :], out_sb[:])

    nc.compile()
    _program_cache = nc
    return nc


def _prep_inputs(x, fm_w, w1, b1, w2, b2, wo, bo):
    x = np.asarray(x, dtype=np.float32)
    fm_w = np.asarray(fm_w, dtype=np.float32)
    w1 = np.asarray(w1, dtype=np.float32)
    w2 = np.asarray(w2, dtype=np.float32)
    wo = np.asarray(wo, dtype=np.float32).reshape(NP + H)
    b1 = np.asarray(b1, dtype=np.float32).reshape(H)
    b2 = np.asarray(b2, dtype=np.float32).reshape(H)
    bo = np.asarray(bo, dtype=np.float32).reshape(1)

    # Scatter the pair weights into the strictly-upper triangle (static
    # index relayout; same (j1, j2>j1) row-major order as the reference).
    wp = np.zeros((N, N), dtype=np.float32)
    wp[_IU1, _IU2] = wo[:NP]

    pack = np.zeros((H, 4), dtype=np.float32)
    pack[:, 0] = b1
    pack[:, 1] = b2
    pack[:, 2] = wo[NP:]
    pack[0, 3] = bo[0]

    bf = ml_dtypes.bfloat16
    wp_bf = np.ascontiguousarray(wp.astype(bf))
    fmwT_bf = np.ascontiguousarray(fm_w.T.astype(bf))   # [4, 512]
    w1_bf = np.ascontiguousarray(w1.astype(bf))
    w2_bf = np.ascontiguousarray(w2.astype(bf))
    xT_bf = np.ascontiguousarray(x.T.astype(bf))        # [512, 512]

    shared = {
        "wp": wp_bf, "fmwT": fmwT_bf, "w1": w1_bf, "w2": w2_bf, "pack": pack,
    }
    in_maps = []
    for c in range(N_CORES):
        m = dict(shared)
        m["xT"] = np.ascontiguousarray(xT_bf[:, c * BC:(c + 1) * BC])
        in_maps.append(m)
    return in_maps


def run(inputs, **spmd_kwargs):
    """Build, run on 8 cores, return (output [512,1] f32, BassKernelResults)."""
    nc = _build_program()
    in_maps = _prep_inputs(**inputs)
    res = run_bass_kernel_spmd(nc, in_maps, list(range(N_CORES)), **spmd_kwargs)
    out = np.concatenate(
        [res.results[c]["out"].reshape(BC) for c in range(N_CORES)]
    ).reshape(B, 1).astype(np.float32)
    return out, res


def kernel(**inputs) -> np.ndarray:
    out, _ = run(inputs)
    return out
